# revision 3
# baseline (speedup 1.0000x reference)
"""Multi-head-free attention (B=4, S=4096, W=512, E=64) on 8 TRN2 NeuronCores.

Sharding: core c handles batch b = c//2, query half h = c%2 (2048 queries).
Each core computes K/V for the full sequence of its batch locally (x[b] is
replicated to the core pair as a host-side pre-transposed bf16 tensor), runs
a flash-style attention over its query half, and writes its [2048, 64] output
slice. No collectives.

Per-core dataflow (all bf16 matmul inputs, fp32 PSUM accumulation):
  x^T [512,4096] --[Wv|Wk] pass--> V^T (p0:64) / K^T (p64:128)
  xq^T [512,2048] --[Wq|Wq] pass--> Q^T duplicated on both partition halves
  K^T replicated to both halves via SBUF->SBUF DMA (for row-packed scores)
  scores: S^T[k,q] = K^T.T @ Q^T, two k-tiles packed in PE row groups (e=64)
  P = exp(S^T * 0.125) on ScalarE, PSUM->SBUF bf16
  Z'^T[e+1,q] += V'.T @ P^T accumulated in PSUM over all k (V' has a ones
  column, so row 64 accumulates the softmax denominator)
  normalize: PE-transpose Z'^T, reciprocal + scale on VectorE, DMA out.
"""

import numpy as np
import ml_dtypes

import concourse.bass as bass
import concourse.mybir as mybir
import concourse.tile as tile
from concourse import bacc
from concourse.bass import ts
from concourse.masks import make_identity
from concourse.bass_utils import run_bass_kernel_spmd

BF16 = mybir.dt.bfloat16
F32 = mybir.dt.float32
NP_BF16 = ml_dtypes.bfloat16

B = 4
S_FULL = 4096
W = 512
E = 64
TQ = 2048  # queries per core
WT = W // 128  # 4 contraction tiles
KT = S_FULL // 128  # 32 key tiles
KP = KT // 2  # 16 key-tile pairs
QC = TQ // 512  # 4 query chunks of 512
SCALE = 0.125  # 1/sqrt(E)

_NC_CACHE = {}


def build_nc():
    nc = bacc.Bacc("TRN2", target_bir_lowering=False)
    xT = nc.dram_tensor("xT", [W, S_FULL], BF16, kind="ExternalInput")
    xq = nc.dram_tensor("xq", [W, TQ], BF16, kind="ExternalInput")
    wqq = nc.dram_tensor("wqq", [W, 128], BF16, kind="ExternalInput")
    wkv = nc.dram_tensor("wkv", [W, 128], BF16, kind="ExternalInput")
    bqq = nc.dram_tensor("bqq", [128, 1], F32, kind="ExternalInput")
    bkv = nc.dram_tensor("bkv", [128, 1], F32, kind="ExternalInput")
    y = nc.dram_tensor("y", [TQ, E], F32, kind="ExternalOutput")

    with tile.TileContext(nc) as tc:
        with (
            tc.tile_pool(name="const", bufs=1) as const,
            tc.tile_pool(name="psZ", bufs=QC, space="PSUM") as psZ,
            tc.tile_pool(name="pp", bufs=3) as ppool,
            tc.tile_pool(name="zsb", bufs=2) as zsbp,
            tc.tile_pool(name="small", bufs=2) as small,
        ):
            # ---- inputs -> SBUF
            xt_sb = const.tile([128, WT, S_FULL], BF16)
            for t in range(WT):
                nc.sync.dma_start(out=xt_sb[:, t, :], in_=xT[t * 128:(t + 1) * 128, :])
            xq_sb = const.tile([128, WT, TQ], BF16)
            for t in range(WT):
                nc.sync.dma_start(out=xq_sb[:, t, :], in_=xq[t * 128:(t + 1) * 128, :])
            wqq_sb = const.tile([128, WT, 128], BF16)
            wkv_sb = const.tile([128, WT, 128], BF16)
            for t in range(WT):
                nc.sync.dma_start(out=wqq_sb[:, t, :], in_=wqq[t * 128:(t + 1) * 128, :])
                nc.sync.dma_start(out=wkv_sb[:, t, :], in_=wkv[t * 128:(t + 1) * 128, :])
            bqq_sb = const.tile([128, 1], F32)
            bkv_sb = const.tile([128, 1], F32)
            nc.sync.dma_start(out=bqq_sb, in_=bqq[:, :])
            nc.sync.dma_start(out=bkv_sb, in_=bkv[:, :])
            ident_bf = const.tile([64, 64], BF16)
            make_identity(nc, ident_bf)
            ident_f32 = const.tile([E + 1, E + 1], F32)
            make_identity(nc, ident_f32)

            ktpair = const.tile([128, S_FULL], BF16)  # K^T on both halves
            qtpair = const.tile([128, TQ], BF16)  # Q^T on both halves
            vt_sb = const.tile([64, S_FULL], BF16)  # V^T
            vp_sb = const.tile([128, KT, E + 1], BF16)  # V' = [V | 1]
            nc.vector.memset(vp_sb, 1.0)

            zps = [
                psZ.tile([E + 1, 512], F32, tag="zacc", name=f"zacc{i}")
                for i in range(QC)
            ]
            out_sb = const.tile([128, 4 * QC, E], F32)

            with tc.tile_pool(name="psA", bufs=3, space="PSUM") as psA:
                # K/V projection over the full sequence
                for ch in range(S_FULL // 512):
                    ps = psA.tile([128, 512], F32, tag="mm")
                    for t in range(WT):
                        nc.tensor.matmul(
                            ps,
                            wkv_sb[:, t, :],
                            xt_sb[:, t, ts(ch, 512)],
                            start=(t == 0),
                            stop=(t == WT - 1),
                        )
                    nc.vector.tensor_scalar_add(
                        vt_sb[:, ts(ch, 512)], ps[0:64, :], bkv_sb[0:64, :]
                    )
                    nc.vector.tensor_scalar_add(
                        ktpair[64:128, ts(ch, 512)], ps[64:128, :], bkv_sb[64:128, :]
                    )
                    nc.sync.dma_start(
                        out=ktpair[0:64, ts(ch, 512)], in_=ktpair[64:128, ts(ch, 512)]
                    )
                # Q projection over this core's query half
                for ch in range(TQ // 512):
                    ps = psA.tile([128, 512], F32, tag="mm")
                    for t in range(WT):
                        nc.tensor.matmul(
                            ps,
                            wqq_sb[:, t, :],
                            xq_sb[:, t, ts(ch, 512)],
                            start=(t == 0),
                            stop=(t == WT - 1),
                        )
                    nc.vector.tensor_scalar_add(qtpair[:, ts(ch, 512)], ps, bqq_sb)
                # V^T -> V tiles (PE transpose), ones column stays from memset
                for kt_i in range(KT):
                    vt_ps = psA.tile([128, E], BF16, tag="mm", name=f"vtps{kt_i}")
                    nc.tensor.transpose(vt_ps, vt_sb[:, ts(kt_i, 128)], ident_bf)
                    nc.vector.tensor_copy(vp_sb[:, kt_i, 0:E], vt_ps)

            with tc.tile_pool(name="psB", bufs=2, space="PSUM") as psB:
                for kp in range(KP):
                    ka, kb = 2 * kp, 2 * kp + 1
                    for qc in range(QC):
                        sp = psB.tile([128, 1024], F32, tag="spair")
                        nc.tensor.matmul(
                            sp[:, 0:512],
                            ktpair[0:64, ts(ka, 128)],
                            qtpair[0:64, ts(qc, 512)],
                            start=True,
                            stop=True,
                        )
                        nc.tensor.matmul(
                            sp[:, 512:1024],
                            ktpair[64:128, ts(kb, 128)],
                            qtpair[64:128, ts(qc, 512)],
                            start=True,
                            stop=True,
                        )
                        p_sb = ppool.tile([128, 1024], BF16, tag="p")
                        nc.scalar.activation(
                            p_sb, sp, mybir.ActivationFunctionType.Exp, scale=SCALE
                        )
                        nc.tensor.matmul(
                            zps[qc],
                            vp_sb[:, ka, :],
                            p_sb[:, 0:512],
                            start=(kp == 0),
                            stop=False,
                        )
                        nc.tensor.matmul(
                            zps[qc],
                            vp_sb[:, kb, :],
                            p_sb[:, 512:1024],
                            start=False,
                            stop=(kp == KP - 1),
                        )

            with tc.tile_pool(name="psT", bufs=2, space="PSUM") as psT:
                for qc in range(QC):
                    zsb = zsbp.tile([E + 1, 512], F32, tag="zsb")
                    nc.vector.tensor_copy(zsb, zps[qc])
                    for sub in range(4):
                        zt = psT.tile([128, E + 1], F32, tag="zt")
                        nc.tensor.transpose(zt, zsb[:, ts(sub, 128)], ident_f32)
                        r = small.tile([128, 1], F32, tag="r")
                        nc.vector.reciprocal(r, zt[:, E:E + 1])
                        nc.vector.tensor_scalar_mul(
                            out_sb[:, qc * 4 + sub, :], zt[:, 0:E], r
                        )
            y_ap = y[:, :].rearrange("(t p) e -> p t e", t=4 * QC)
            nc.sync.dma_start(out=y_ap, in_=out_sb)
    nc.compile()
    return nc


def get_nc():
    if "nc" not in _NC_CACHE:
        _NC_CACHE["nc"] = build_nc()
    return _NC_CACHE["nc"]


def kernel(x, Wq, bq, Wk, bk, Wv, bv, **_unused):
    x = np.asarray(x, dtype=np.float32)
    Wq = np.asarray(Wq, dtype=np.float32)
    Wk = np.asarray(Wk, dtype=np.float32)
    Wv = np.asarray(Wv, dtype=np.float32)
    bq = np.asarray(bq, dtype=np.float32)
    bk = np.asarray(bk, dtype=np.float32)
    bv = np.asarray(bv, dtype=np.float32)

    # host-side layout prep (sharding): per-batch transposed bf16 activations,
    # packed projection weights [Wv.T | Wk.T] and [Wq.T | Wq.T]
    wkv_host = np.ascontiguousarray(
        np.concatenate([Wv.T, Wk.T], axis=1)
    ).astype(NP_BF16)
    wqq_host = np.ascontiguousarray(
        np.concatenate([Wq.T, Wq.T], axis=1)
    ).astype(NP_BF16)
    bkv_host = np.ascontiguousarray(
        np.concatenate([bv, bk]).reshape(128, 1)
    ).astype(np.float32)
    bqq_host = np.ascontiguousarray(
        np.concatenate([bq, bq]).reshape(128, 1)
    ).astype(np.float32)

    xT_host = [
        np.ascontiguousarray(x[b].T).astype(NP_BF16) for b in range(B)
    ]

    in_maps = []
    for c in range(8):
        b, h = c // 2, c % 2
        xT_b = xT_host[b]
        in_maps.append(
            {
                "xT": xT_b,
                "xq": np.ascontiguousarray(xT_b[:, h * TQ:(h + 1) * TQ]),
                "wqq": wqq_host,
                "wkv": wkv_host,
                "bqq": bqq_host,
                "bkv": bkv_host,
            }
        )

    nc = get_nc()
    res = run_bass_kernel_spmd(nc, in_maps, core_ids=list(range(8)))

    out = np.empty((B, S_FULL, E), dtype=np.float32)
    for c in range(8):
        b, h = c // 2, c % 2
        out[b, h * TQ:(h + 1) * TQ, :] = res.results[c]["y"]
    return out


# revision 4
# speedup vs baseline: 1.0240x; 1.0240x over previous
"""Attention (B=4, S=4096, W=512, E=64) on 8 TRN2 NeuronCores.

Sharding: core c handles batch b = c//2, query half h = c%2 (2048 queries).
Each core receives x[b]^T as bf16 with the key/value columns ordered so that
this core's query half occupies columns [0, 2048) (softmax over keys is
permutation invariant as long as K and V share the order, so odd cores get
the two halves swapped). K/V are computed for the full sequence locally; a
flash-style attention runs over the core's query half. No collectives.

Per-core dataflow (bf16 matmul inputs, fp32 PSUM accumulation):
  x^T [512,4096] --[Wv|Wk] pass--> V^T (p0:64) / K^T (p64:128)
  x^T[:, :2048] --[Wq|Wq] pass--> Q^T duplicated on both partition halves
  K^T replicated to both halves via SBUF->SBUF DMA (for row-packed scores)
  scores: S^T[k,q] = K^T.T @ Q^T, two k-tiles packed in PE row groups (e=64)
  P = exp(S^T * 0.125) on ScalarE (PSUM -> SBUF bf16)
  Z'^T[e+1,q] += V'.T @ P^T accumulated in PSUM over all k (V' carries a
  ones column, so row 64 accumulates the softmax denominator)
  normalize: PE-transpose Z'^T, reciprocal + scale on VectorE, DMA out.
"""

import numpy as np
import ml_dtypes

import concourse.bass as bass
import concourse.mybir as mybir
import concourse.tile as tile
from concourse import bacc
from concourse.bass import ts
from concourse.masks import make_identity
from concourse.bass_utils import run_bass_kernel_spmd

BF16 = mybir.dt.bfloat16
F32 = mybir.dt.float32
NP_BF16 = ml_dtypes.bfloat16

B = 4
S_FULL = 4096
W = 512
E = 64
TQ = 2048  # queries per core
WT = W // 128  # 4 contraction tiles
KT = S_FULL // 128  # 32 key tiles
KP = KT // 2  # 16 key-tile pairs
QC = TQ // 512  # 4 query chunks of 512
NCH = S_FULL // 512  # 8 projection chunks
SCALE = 0.125  # 1/sqrt(E)

_NC_CACHE = {}


def build_nc():
    nc = bacc.Bacc("TRN2", target_bir_lowering=False)
    xT = nc.dram_tensor("xT", [W, S_FULL], BF16, kind="ExternalInput")
    wqq = nc.dram_tensor("wqq", [W, 128], BF16, kind="ExternalInput")
    wkv = nc.dram_tensor("wkv", [W, 128], BF16, kind="ExternalInput")
    bqq = nc.dram_tensor("bqq", [128, 1], F32, kind="ExternalInput")
    bkv = nc.dram_tensor("bkv", [128, 1], F32, kind="ExternalInput")
    y = nc.dram_tensor("y", [TQ, E], F32, kind="ExternalOutput")

    with tile.TileContext(nc) as tc:
        with (
            tc.tile_pool(name="const", bufs=1) as const,
            tc.tile_pool(name="psZ", bufs=QC, space="PSUM") as psZ,
            tc.tile_pool(name="pp", bufs=4) as ppool,
            tc.tile_pool(name="zsb", bufs=2) as zsbp,
            tc.tile_pool(name="small", bufs=2) as small,
            tc.tile_pool(name="outp", bufs=2) as outp,
        ):
            # ---- small inputs first so compute can start immediately
            wqq_sb = const.tile([128, WT, 128], BF16)
            wkv_sb = const.tile([128, WT, 128], BF16)
            for t in range(WT):
                nc.sync.dma_start(out=wkv_sb[:, t, :], in_=wkv[t * 128:(t + 1) * 128, :])
                nc.sync.dma_start(out=wqq_sb[:, t, :], in_=wqq[t * 128:(t + 1) * 128, :])
            bqq_sb = const.tile([128, 1], F32)
            bkv_sb = const.tile([128, 1], F32)
            nc.sync.dma_start(out=bkv_sb, in_=bkv[:, :])
            nc.sync.dma_start(out=bqq_sb, in_=bqq[:, :])
            ident_bf = const.tile([64, 64], BF16)
            make_identity(nc, ident_bf)
            ident_f32 = const.tile([E + 1, E + 1], F32)
            make_identity(nc, ident_f32)

            # x^T streamed in per 512-column chunk ([128, 512] blocks)
            xt_sb = const.tile([128, WT, S_FULL], BF16)
            for ch in range(NCH):
                for t in range(WT):
                    nc.sync.dma_start(
                        out=xt_sb[:, t, ts(ch, 512)],
                        in_=xT[t * 128:(t + 1) * 128, ts(ch, 512)],
                    )

            ktpair = const.tile([128, S_FULL], BF16)  # K^T on both halves
            qtpair = const.tile([128, TQ], BF16)  # Q^T on both halves
            vt_sb = const.tile([64, S_FULL], BF16)  # V^T
            vp_sb = const.tile([128, KT, E + 1], BF16)  # V' = [V | 1]
            nc.vector.memset(vp_sb, 1.0)

            zps = [
                psZ.tile([E + 1, 512], F32, tag="zacc", name=f"zacc{i}")
                for i in range(QC)
            ]

            with tc.tile_pool(name="psA", bufs=3, space="PSUM") as psA:
                for ch in range(NCH):
                    # K/V projection chunk
                    ps = psA.tile([128, 512], F32, tag="mm", name=f"pskv{ch}")
                    for t in range(WT):
                        nc.tensor.matmul(
                            ps,
                            wkv_sb[:, t, :],
                            xt_sb[:, t, ts(ch, 512)],
                            start=(t == 0),
                            stop=(t == WT - 1),
                        )
                    nc.vector.tensor_scalar_add(
                        vt_sb[:, ts(ch, 512)], ps[0:64, :], bkv_sb[0:64, :]
                    )
                    nc.vector.tensor_scalar_add(
                        ktpair[64:128, ts(ch, 512)], ps[64:128, :], bkv_sb[64:128, :]
                    )
                    nc.sync.dma_start(
                        out=ktpair[0:64, ts(ch, 512)], in_=ktpair[64:128, ts(ch, 512)]
                    )
                    # Q projection chunk (queries live in columns [0, 2048))
                    if ch < QC:
                        psq = psA.tile([128, 512], F32, tag="mm", name=f"psq{ch}")
                        for t in range(WT):
                            nc.tensor.matmul(
                                psq,
                                wqq_sb[:, t, :],
                                xt_sb[:, t, ts(ch, 512)],
                                start=(t == 0),
                                stop=(t == WT - 1),
                            )
                        nc.vector.tensor_scalar_add(
                            qtpair[:, ts(ch, 512)], psq, bqq_sb
                        )
                    # V^T -> V' tiles for this chunk (ones column from memset)
                    for kt_i in range(4 * ch, 4 * ch + 4):
                        vt_ps = psA.tile([128, E], BF16, tag="mm", name=f"vtps{kt_i}")
                        nc.tensor.transpose(vt_ps, vt_sb[:, ts(kt_i, 128)], ident_bf)
                        nc.vector.tensor_copy(vp_sb[:, kt_i, 0:E], vt_ps)

            with tc.tile_pool(name="psB", bufs=2, space="PSUM") as psB:
                for kp in range(KP):
                    ka, kb = 2 * kp, 2 * kp + 1
                    for qc in range(QC):
                        sp = psB.tile(
                            [128, 1024], F32, tag="spair", name=f"sp{kp}_{qc}"
                        )
                        nc.tensor.matmul(
                            sp[:, 0:512],
                            ktpair[0:64, ts(ka, 128)],
                            qtpair[0:64, ts(qc, 512)],
                            start=True,
                            stop=True,
                        )
                        nc.tensor.matmul(
                            sp[:, 512:1024],
                            ktpair[64:128, ts(kb, 128)],
                            qtpair[64:128, ts(qc, 512)],
                            start=True,
                            stop=True,
                        )
                        p_sb = ppool.tile(
                            [128, 1024], BF16, tag="p", name=f"p{kp}_{qc}"
                        )
                        nc.scalar.activation(
                            p_sb, sp, mybir.ActivationFunctionType.Exp, scale=SCALE
                        )
                        nc.tensor.matmul(
                            zps[qc],
                            vp_sb[:, ka, :],
                            p_sb[:, 0:512],
                            start=(kp == 0),
                            stop=False,
                        )
                        nc.tensor.matmul(
                            zps[qc],
                            vp_sb[:, kb, :],
                            p_sb[:, 512:1024],
                            start=False,
                            stop=(kp == KP - 1),
                        )

            with tc.tile_pool(name="psT", bufs=2, space="PSUM") as psT:
                for qc in range(QC):
                    zsb = zsbp.tile([E + 1, 512], F32, tag="zsb", name=f"zsb{qc}")
                    nc.vector.tensor_copy(zsb, zps[qc])
                    o_sb = outp.tile([128, 4, E], F32, tag="o", name=f"osb{qc}")
                    for sub in range(4):
                        zt = psT.tile([128, E + 1], F32, tag="zt", name=f"zt{qc}_{sub}")
                        nc.tensor.transpose(zt, zsb[:, ts(sub, 128)], ident_f32)
                        r = small.tile([128, 1], F32, tag="r", name=f"r{qc}_{sub}")
                        nc.vector.reciprocal(r, zt[:, E:E + 1])
                        nc.vector.tensor_scalar_mul(o_sb[:, sub, :], zt[:, 0:E], r)
                    y_ap = y[ts(qc, 512), :].rearrange("(t p) e -> p t e", t=4)
                    nc.sync.dma_start(out=y_ap, in_=o_sb)
    nc.compile()
    return nc


def get_nc():
    if "nc" not in _NC_CACHE:
        _NC_CACHE["nc"] = build_nc()
    return _NC_CACHE["nc"]


def make_in_maps(x, Wq, bq, Wk, bk, Wv, bv):
    x = np.asarray(x, dtype=np.float32)
    Wq = np.asarray(Wq, dtype=np.float32)
    Wk = np.asarray(Wk, dtype=np.float32)
    Wv = np.asarray(Wv, dtype=np.float32)
    bq = np.asarray(bq, dtype=np.float32)
    bk = np.asarray(bk, dtype=np.float32)
    bv = np.asarray(bv, dtype=np.float32)

    wkv_host = np.ascontiguousarray(
        np.concatenate([Wv.T, Wk.T], axis=1)
    ).astype(NP_BF16)
    wqq_host = np.ascontiguousarray(
        np.concatenate([Wq.T, Wq.T], axis=1)
    ).astype(NP_BF16)
    bkv_host = np.ascontiguousarray(
        np.concatenate([bv, bk]).reshape(128, 1)
    ).astype(np.float32)
    bqq_host = np.ascontiguousarray(
        np.concatenate([bq, bq]).reshape(128, 1)
    ).astype(np.float32)

    in_maps = []
    for c in range(8):
        b, h = c // 2, c % 2
        xT_b = np.asarray(x[b].T, dtype=NP_BF16)
        if h == 1:  # put this core's query half into columns [0, 2048)
            xT_b = np.concatenate([xT_b[:, TQ:], xT_b[:, :TQ]], axis=1)
        in_maps.append(
            {
                "xT": np.ascontiguousarray(xT_b),
                "wqq": wqq_host,
                "wkv": wkv_host,
                "bqq": bqq_host,
                "bkv": bkv_host,
            }
        )
    return in_maps


def assemble(results):
    out = np.empty((B, S_FULL, E), dtype=np.float32)
    for c in range(8):
        b, h = c // 2, c % 2
        out[b, h * TQ:(h + 1) * TQ, :] = results[c]["y"]
    return out


def kernel(x, Wq, bq, Wk, bk, Wv, bv, **_unused):
    in_maps = make_in_maps(x, Wq, bq, Wk, bk, Wv, bv)
    nc = get_nc()
    res = run_bass_kernel_spmd(nc, in_maps, core_ids=list(range(8)))
    return assemble(res.results)


# revision 5
# speedup vs baseline: 1.2828x; 1.2528x over previous
"""Attention (B=4, S=4096, W=512, E=64) on 8 TRN2 NeuronCores.

Sharding: core c handles batch b = c//2, query half h = c%2 (2048 queries).
Each core receives x[b]^T as bf16 with the key/value columns ordered so that
this core's query half occupies columns [0, 2048) (softmax over keys is
permutation invariant as long as K and V share the order, so odd cores get
the two halves swapped). K/V are computed for the full sequence locally; a
flash-style attention runs over the core's query half. No collectives.

Per-core dataflow (bf16 matmul inputs, fp32 PSUM accumulation):
  x^T [512,4096] --[Wv|Wk] pass--> V^T (p0:64) / K^T (p64:128)
  x^T[:, :2048] --[Wq|Wq] pass--> Q^T duplicated on both partition halves
  K^T replicated to both halves via SWDGE SBUF->SBUF DMA (row-packed scores)
  scores: S^T[k,q] = K^T.T @ Q^T, two k-tiles packed in PE row groups (e=64)
  P = exp(S^T * 0.125) on ScalarE (PSUM -> SBUF bf16)
  Z'^T[e+1,q] += V'.T @ P^T accumulated in PSUM over all k (V' carries a
  ones column, so row 64 accumulates the softmax denominator)
  normalize per query chunk: PE-transpose Z'^T, reciprocal + scale on
  VectorE, DMA out — overlapped with the next chunk's score sweep.
"""

import numpy as np
import ml_dtypes

import concourse.bass as bass
import concourse.mybir as mybir
import concourse.tile as tile
from concourse import bacc
from concourse.bass import ts
from concourse.masks import make_identity
from concourse.bass_utils import run_bass_kernel_spmd

BF16 = mybir.dt.bfloat16
F32 = mybir.dt.float32
NP_BF16 = ml_dtypes.bfloat16

B = 4
S_FULL = 4096
W = 512
E = 64
TQ = 2048  # queries per core
WT = W // 128  # 4 contraction tiles
KT = S_FULL // 128  # 32 key tiles
KP = KT // 2  # 16 key-tile pairs
QC = TQ // 512  # 4 query chunks of 512
NCH = S_FULL // 512  # 8 projection chunks
SCALE = 0.125  # 1/sqrt(E)

_NC_CACHE = {}


def build_nc():
    nc = bacc.Bacc("TRN2", target_bir_lowering=False)
    xT = nc.dram_tensor("xT", [W, S_FULL], BF16, kind="ExternalInput")
    wqq = nc.dram_tensor("wqq", [W, 128], BF16, kind="ExternalInput")
    wkv = nc.dram_tensor("wkv", [W, 128], BF16, kind="ExternalInput")
    bqq = nc.dram_tensor("bqq", [128, 1], F32, kind="ExternalInput")
    bkv = nc.dram_tensor("bkv", [128, 1], F32, kind="ExternalInput")
    y = nc.dram_tensor("y", [TQ, E], F32, kind="ExternalOutput")

    with tile.TileContext(nc) as tc:
        with (
            tc.tile_pool(name="const", bufs=1) as const,
            tc.tile_pool(name="psZ", bufs=2, space="PSUM") as psZ,
            tc.tile_pool(name="pp", bufs=4) as ppool,
            tc.tile_pool(name="zsb", bufs=2) as zsbp,
            tc.tile_pool(name="small", bufs=2) as small,
            tc.tile_pool(name="outp", bufs=2) as outp,
        ):
            # ---- small inputs via SWDGE (gpsimd) to keep the HWDGE queue
            # free for the big x^T stream
            wqq_sb = const.tile([128, WT, 128], BF16)
            wkv_sb = const.tile([128, WT, 128], BF16)
            for t in range(WT):
                nc.gpsimd.dma_start(
                    out=wkv_sb[:, t, :], in_=wkv[t * 128:(t + 1) * 128, :]
                )
                nc.gpsimd.dma_start(
                    out=wqq_sb[:, t, :], in_=wqq[t * 128:(t + 1) * 128, :]
                )
            bqq_sb = const.tile([128, 1], F32)
            bkv_sb = const.tile([128, 1], F32)
            nc.gpsimd.dma_start(out=bkv_sb, in_=bkv[:, :])
            nc.gpsimd.dma_start(out=bqq_sb, in_=bqq[:, :])
            ident_bf = const.tile([64, 64], BF16)
            make_identity(nc, ident_bf)
            ident_f32 = const.tile([E + 1, E + 1], F32)
            make_identity(nc, ident_f32)

            # x^T streamed in per 512-column chunk ([128, 512] blocks, HWDGE)
            xt_sb = const.tile([128, WT, S_FULL], BF16)
            for ch in range(NCH):
                for t in range(WT):
                    nc.sync.dma_start(
                        out=xt_sb[:, t, ts(ch, 512)],
                        in_=xT[t * 128:(t + 1) * 128, ts(ch, 512)],
                    )

            ktpair = const.tile([128, S_FULL], BF16)  # K^T on both halves
            qtpair = const.tile([128, TQ], BF16)  # Q^T on both halves
            vt_sb = const.tile([64, S_FULL], BF16)  # V^T
            vp_sb = const.tile([128, KT, E + 1], BF16)  # V' = [V | 1]
            nc.vector.memset(vp_sb, 1.0)

            with tc.tile_pool(name="psA", bufs=3, space="PSUM") as psA:
                for ch in range(NCH):
                    # K/V projection chunk
                    ps = psA.tile([128, 512], F32, tag="mm", name=f"pskv{ch}")
                    for t in range(WT):
                        nc.tensor.matmul(
                            ps,
                            wkv_sb[:, t, :],
                            xt_sb[:, t, ts(ch, 512)],
                            start=(t == 0),
                            stop=(t == WT - 1),
                        )
                    nc.vector.tensor_scalar_add(
                        vt_sb[:, ts(ch, 512)], ps[0:64, :], bkv_sb[0:64, :]
                    )
                    nc.vector.tensor_scalar_add(
                        ktpair[64:128, ts(ch, 512)], ps[64:128, :], bkv_sb[64:128, :]
                    )
                    nc.gpsimd.dma_start(
                        out=ktpair[0:64, ts(ch, 512)], in_=ktpair[64:128, ts(ch, 512)]
                    )
                    # Q projection chunk (queries live in columns [0, 2048))
                    if ch < QC:
                        psq = psA.tile([128, 512], F32, tag="mm", name=f"psq{ch}")
                        for t in range(WT):
                            nc.tensor.matmul(
                                psq,
                                wqq_sb[:, t, :],
                                xt_sb[:, t, ts(ch, 512)],
                                start=(t == 0),
                                stop=(t == WT - 1),
                            )
                        nc.vector.tensor_scalar_add(
                            qtpair[:, ts(ch, 512)], psq, bqq_sb
                        )
                    # V^T -> V' tiles for this chunk (ones column from memset)
                    for kt_i in range(4 * ch, 4 * ch + 4):
                        vt_ps = psA.tile([128, E], BF16, tag="mm", name=f"vtps{kt_i}")
                        nc.tensor.transpose(vt_ps, vt_sb[:, ts(kt_i, 128)], ident_bf)
                        nc.vector.tensor_copy(vp_sb[:, kt_i, 0:E], vt_ps)

            with (
                tc.tile_pool(name="psB", bufs=2, space="PSUM") as psB,
                tc.tile_pool(name="psT", bufs=2, space="PSUM") as psT,
            ):
                for qc in range(QC):
                    zp = psZ.tile([E + 1, 512], F32, tag="zacc", name=f"zacc{qc}")
                    for kp in range(KP):
                        ka, kb = 2 * kp, 2 * kp + 1
                        sp = psB.tile(
                            [128, 1024], F32, tag="spair", name=f"sp{qc}_{kp}"
                        )
                        nc.tensor.matmul(
                            sp[:, 0:512],
                            ktpair[0:64, ts(ka, 128)],
                            qtpair[0:64, ts(qc, 512)],
                            start=True,
                            stop=True,
                        )
                        nc.tensor.matmul(
                            sp[:, 512:1024],
                            ktpair[64:128, ts(kb, 128)],
                            qtpair[64:128, ts(qc, 512)],
                            start=True,
                            stop=True,
                        )
                        p_sb = ppool.tile(
                            [128, 1024], BF16, tag="p", name=f"p{qc}_{kp}"
                        )
                        nc.scalar.activation(
                            p_sb, sp, mybir.ActivationFunctionType.Exp, scale=SCALE
                        )
                        nc.tensor.matmul(
                            zp,
                            vp_sb[:, ka, :],
                            p_sb[:, 0:512],
                            start=(kp == 0),
                            stop=False,
                        )
                        nc.tensor.matmul(
                            zp,
                            vp_sb[:, kb, :],
                            p_sb[:, 512:1024],
                            start=False,
                            stop=(kp == KP - 1),
                        )
                    # normalize this query chunk (overlaps next chunk's sweep)
                    zsb = zsbp.tile([E + 1, 512], F32, tag="zsb", name=f"zsb{qc}")
                    nc.vector.tensor_copy(zsb, zp)
                    o_sb = outp.tile([128, 4, E], F32, tag="o", name=f"osb{qc}")
                    for sub in range(4):
                        zt = psT.tile(
                            [128, E + 1], F32, tag="zt", name=f"zt{qc}_{sub}"
                        )
                        nc.tensor.transpose(zt, zsb[:, ts(sub, 128)], ident_f32)
                        r = small.tile([128, 1], F32, tag="r", name=f"r{qc}_{sub}")
                        nc.vector.reciprocal(r, zt[:, E:E + 1])
                        nc.vector.tensor_scalar_mul(
                            o_sb[:, sub, :], zt[:, 0:E], r
                        )
                    y_ap = y[ts(qc, 512), :].rearrange("(t p) e -> p t e", t=4)
                    nc.gpsimd.dma_start(out=y_ap, in_=o_sb)
    nc.compile()
    return nc


def get_nc():
    if "nc" not in _NC_CACHE:
        _NC_CACHE["nc"] = build_nc()
    return _NC_CACHE["nc"]


def make_in_maps(x, Wq, bq, Wk, bk, Wv, bv):
    x = np.asarray(x, dtype=np.float32)
    Wq = np.asarray(Wq, dtype=np.float32)
    Wk = np.asarray(Wk, dtype=np.float32)
    Wv = np.asarray(Wv, dtype=np.float32)
    bq = np.asarray(bq, dtype=np.float32)
    bk = np.asarray(bk, dtype=np.float32)
    bv = np.asarray(bv, dtype=np.float32)

    wkv_host = np.ascontiguousarray(
        np.concatenate([Wv.T, Wk.T], axis=1)
    ).astype(NP_BF16)
    wqq_host = np.ascontiguousarray(
        np.concatenate([Wq.T, Wq.T], axis=1)
    ).astype(NP_BF16)
    bkv_host = np.ascontiguousarray(
        np.concatenate([bv, bk]).reshape(128, 1)
    ).astype(np.float32)
    bqq_host = np.ascontiguousarray(
        np.concatenate([bq, bq]).reshape(128, 1)
    ).astype(np.float32)

    in_maps = []
    for c in range(8):
        b, h = c // 2, c % 2
        xT_b = np.asarray(x[b].T, dtype=NP_BF16)
        if h == 1:  # put this core's query half into columns [0, 2048)
            xT_b = np.concatenate([xT_b[:, TQ:], xT_b[:, :TQ]], axis=1)
        in_maps.append(
            {
                "xT": np.ascontiguousarray(xT_b),
                "wqq": wqq_host,
                "wkv": wkv_host,
                "bqq": bqq_host,
                "bkv": bkv_host,
            }
        )
    return in_maps


def assemble(results):
    out = np.empty((B, S_FULL, E), dtype=np.float32)
    for c in range(8):
        b, h = c // 2, c % 2
        out[b, h * TQ:(h + 1) * TQ, :] = results[c]["y"]
    return out


def kernel(x, Wq, bq, Wk, bk, Wv, bv, **_unused):
    in_maps = make_in_maps(x, Wq, bq, Wk, bk, Wv, bv)
    nc = get_nc()
    res = run_bass_kernel_spmd(nc, in_maps, core_ids=list(range(8)))
    return assemble(res.results)


# revision 6
# speedup vs baseline: 1.3518x; 1.0538x over previous
"""Attention (B=4, S=4096, W=512, E=64) on 8 TRN2 NeuronCores.

Sharding: core c handles batch b = c//2, query half h = c%2 (2048 queries).
Each core receives x[b]^T as bf16 with the key/value columns ordered so that
this core's query half occupies columns [0, 2048) (softmax over keys is
permutation invariant as long as K and V share the order, so odd cores get
the two halves swapped). K/V are computed for the full sequence locally; a
flash-style attention runs over the core's query half. No collectives.

Per-core dataflow (bf16 matmul inputs, fp32 PSUM accumulation):
  x^T [512,4096] --[Wv|Wk] pass--> kv = V^T (p0:64) / K^T (p64:128)
  x^T[:, :2048] --[Wq|Wq] pass--> Q^T duplicated on both partition halves
  K^T replicated to partitions 0:64 via SWDGE SBUF->SBUF DMA (row packing)
  scores: S^T[k,q] = K^T.T @ Q^T, two k-tiles packed in PE row groups (e=64)
  P = exp(S^T * 0.125) on ScalarE (PSUM -> SBUF bf16)
  Z'^T[e+1,q] += V'.T @ P^T accumulated in PSUM over all k (V' carries a
  ones column, so row 64 accumulates the softmax denominator)
  normalize per query chunk: PE-transpose Z'^T, reciprocal + scale on
  VectorE, DMA out — overlapped with the next chunk's score sweep.
"""

import numpy as np
import ml_dtypes

import concourse.bass as bass
import concourse.mybir as mybir
import concourse.tile as tile
from concourse import bacc
from concourse.bass import ts
from concourse.masks import make_identity
from concourse.bass_utils import run_bass_kernel_spmd

BF16 = mybir.dt.bfloat16
F32 = mybir.dt.float32
NP_BF16 = ml_dtypes.bfloat16

B = 4
S_FULL = 4096
W = 512
E = 64
TQ = 2048  # queries per core
WT = W // 128  # 4 contraction tiles
KT = S_FULL // 128  # 32 key tiles
KP = KT // 2  # 16 key-tile pairs
QC = TQ // 512  # 4 query chunks of 512
NCH = S_FULL // 512  # 8 projection chunks
SCALE = 0.125  # 1/sqrt(E)

_NC_CACHE = {}


def build_nc():
    nc = bacc.Bacc("TRN2", target_bir_lowering=False)
    xT = nc.dram_tensor("xT", [W, S_FULL], BF16, kind="ExternalInput")
    wqq = nc.dram_tensor("wqq", [W, 128], BF16, kind="ExternalInput")
    wkv = nc.dram_tensor("wkv", [W, 128], BF16, kind="ExternalInput")
    bqq = nc.dram_tensor("bqq", [128, 1], F32, kind="ExternalInput")
    bkv = nc.dram_tensor("bkv", [128, 1], F32, kind="ExternalInput")
    y = nc.dram_tensor("y", [TQ, E], F32, kind="ExternalOutput")

    with tile.TileContext(nc) as tc:
        with (
            tc.tile_pool(name="const", bufs=1) as const,
            tc.tile_pool(name="psZ", bufs=2, space="PSUM") as psZ,
            tc.tile_pool(name="pp", bufs=4) as ppool,
            tc.tile_pool(name="zsb", bufs=2) as zsbp,
            tc.tile_pool(name="small", bufs=2) as small,
            tc.tile_pool(name="outp", bufs=2) as outp,
        ):
            # weights/biases as single HWDGE DMAs ahead of the x^T stream
            wqq_sb = const.tile([128, WT, 128], BF16)
            wkv_sb = const.tile([128, WT, 128], BF16)
            nc.sync.dma_start(
                out=wkv_sb, in_=wkv[:, :].rearrange("(t p) m -> p t m", t=WT)
            )
            nc.sync.dma_start(
                out=wqq_sb, in_=wqq[:, :].rearrange("(t p) m -> p t m", t=WT)
            )
            bqq_sb = const.tile([128, 1], F32)
            bkv_sb = const.tile([128, 1], F32)
            nc.sync.dma_start(out=bkv_sb, in_=bkv[:, :])
            nc.sync.dma_start(out=bqq_sb, in_=bqq[:, :])

            # x^T streamed in per 512-column chunk ([128, 512] blocks, HWDGE)
            xt_sb = const.tile([128, WT, S_FULL], BF16)
            for ch in range(NCH):
                for t in range(WT):
                    nc.sync.dma_start(
                        out=xt_sb[:, t, ts(ch, 512)],
                        in_=xT[t * 128:(t + 1) * 128, ts(ch, 512)],
                    )

            ident_bf = const.tile([64, 64], BF16)
            make_identity(nc, ident_bf)
            ident_f32 = const.tile([E + 1, E + 1], F32)
            make_identity(nc, ident_f32)

            kv_sb = const.tile([128, S_FULL], BF16)  # V^T (p0:64) / K^T (p64:)
            krep = const.tile([64, S_FULL], BF16)  # K^T replica on p0:64
            qtpair = const.tile([128, TQ], BF16)  # Q^T on both halves
            vp_sb = const.tile([128, KT, E + 1], BF16)  # V' = [V | 1]
            nc.vector.memset(vp_sb, 1.0)

            with tc.tile_pool(name="psA", bufs=4, space="PSUM") as psA:
                for ch in range(NCH):
                    # K/V projection chunk; one fused bias add on VectorE
                    ps = psA.tile([128, 512], F32, tag="mm", name=f"pskv{ch}")
                    for t in range(WT):
                        nc.tensor.matmul(
                            ps,
                            wkv_sb[:, t, :],
                            xt_sb[:, t, ts(ch, 512)],
                            start=(t == 0),
                            stop=(t == WT - 1),
                        )
                    nc.vector.tensor_scalar_add(
                        kv_sb[:, ts(ch, 512)], ps, bkv_sb
                    )
                    nc.gpsimd.dma_start(
                        out=krep[:, ts(ch, 512)], in_=kv_sb[64:128, ts(ch, 512)]
                    )
                    # Q projection chunk (queries live in columns [0, 2048))
                    if ch < QC:
                        psq = psA.tile([128, 512], F32, tag="mm", name=f"psq{ch}")
                        for t in range(WT):
                            nc.tensor.matmul(
                                psq,
                                wqq_sb[:, t, :],
                                xt_sb[:, t, ts(ch, 512)],
                                start=(t == 0),
                                stop=(t == WT - 1),
                            )
                        nc.vector.tensor_scalar_add(
                            qtpair[:, ts(ch, 512)], psq, bqq_sb
                        )
                    # V^T -> V' tiles; copies on the idle ScalarE
                    for kt_i in range(4 * ch, 4 * ch + 4):
                        vt_ps = psA.tile([128, E], BF16, tag="mm", name=f"vtps{kt_i}")
                        nc.tensor.transpose(
                            vt_ps, kv_sb[0:64, ts(kt_i, 128)], ident_bf
                        )
                        nc.scalar.copy(vp_sb[:, kt_i, 0:E], vt_ps)

            with (
                tc.tile_pool(name="psB", bufs=2, space="PSUM") as psB,
                tc.tile_pool(name="psT", bufs=2, space="PSUM") as psT,
            ):
                for qc in range(QC):
                    zp = psZ.tile([E + 1, 512], F32, tag="zacc", name=f"zacc{qc}")
                    for kp in range(KP):
                        ka, kb = 2 * kp, 2 * kp + 1
                        sp = psB.tile(
                            [128, 1024], F32, tag="spair", name=f"sp{qc}_{kp}"
                        )
                        nc.tensor.matmul(
                            sp[:, 0:512],
                            krep[:, ts(ka, 128)],
                            qtpair[0:64, ts(qc, 512)],
                            start=True,
                            stop=True,
                        )
                        nc.tensor.matmul(
                            sp[:, 512:1024],
                            kv_sb[64:128, ts(kb, 128)],
                            qtpair[64:128, ts(qc, 512)],
                            start=True,
                            stop=True,
                        )
                        p_sb = ppool.tile(
                            [128, 1024], BF16, tag="p", name=f"p{qc}_{kp}"
                        )
                        nc.scalar.activation(
                            p_sb, sp, mybir.ActivationFunctionType.Exp, scale=SCALE
                        )
                        nc.tensor.matmul(
                            zp,
                            vp_sb[:, ka, :],
                            p_sb[:, 0:512],
                            start=(kp == 0),
                            stop=False,
                        )
                        nc.tensor.matmul(
                            zp,
                            vp_sb[:, kb, :],
                            p_sb[:, 512:1024],
                            start=False,
                            stop=(kp == KP - 1),
                        )
                    # normalize this query chunk (overlaps next chunk's sweep)
                    zsb = zsbp.tile([E + 1, 512], F32, tag="zsb", name=f"zsb{qc}")
                    nc.vector.tensor_copy(zsb, zp)
                    o_sb = outp.tile([128, 4, E], F32, tag="o", name=f"osb{qc}")
                    for sub in range(4):
                        zt = psT.tile(
                            [128, E + 1], F32, tag="zt", name=f"zt{qc}_{sub}"
                        )
                        nc.tensor.transpose(zt, zsb[:, ts(sub, 128)], ident_f32)
                        r = small.tile([128, 1], F32, tag="r", name=f"r{qc}_{sub}")
                        nc.vector.reciprocal(r, zt[:, E:E + 1])
                        nc.vector.tensor_scalar_mul(
                            o_sb[:, sub, :], zt[:, 0:E], r
                        )
                    y_ap = y[ts(qc, 512), :].rearrange("(t p) e -> p t e", t=4)
                    nc.gpsimd.dma_start(out=y_ap, in_=o_sb)
    nc.compile()
    return nc


def get_nc():
    if "nc" not in _NC_CACHE:
        _NC_CACHE["nc"] = build_nc()
    return _NC_CACHE["nc"]


def make_in_maps(x, Wq, bq, Wk, bk, Wv, bv):
    x = np.asarray(x, dtype=np.float32)
    Wq = np.asarray(Wq, dtype=np.float32)
    Wk = np.asarray(Wk, dtype=np.float32)
    Wv = np.asarray(Wv, dtype=np.float32)
    bq = np.asarray(bq, dtype=np.float32)
    bk = np.asarray(bk, dtype=np.float32)
    bv = np.asarray(bv, dtype=np.float32)

    wkv_host = np.ascontiguousarray(
        np.concatenate([Wv.T, Wk.T], axis=1)
    ).astype(NP_BF16)
    wqq_host = np.ascontiguousarray(
        np.concatenate([Wq.T, Wq.T], axis=1)
    ).astype(NP_BF16)
    bkv_host = np.ascontiguousarray(
        np.concatenate([bv, bk]).reshape(128, 1)
    ).astype(np.float32)
    bqq_host = np.ascontiguousarray(
        np.concatenate([bq, bq]).reshape(128, 1)
    ).astype(np.float32)

    in_maps = []
    for c in range(8):
        b, h = c // 2, c % 2
        xT_b = np.asarray(x[b].T, dtype=NP_BF16)
        if h == 1:  # put this core's query half into columns [0, 2048)
            xT_b = np.concatenate([xT_b[:, TQ:], xT_b[:, :TQ]], axis=1)
        in_maps.append(
            {
                "xT": np.ascontiguousarray(xT_b),
                "wqq": wqq_host,
                "wkv": wkv_host,
                "bqq": bqq_host,
                "bkv": bkv_host,
            }
        )
    return in_maps


def assemble(results):
    out = np.empty((B, S_FULL, E), dtype=np.float32)
    for c in range(8):
        b, h = c // 2, c % 2
        out[b, h * TQ:(h + 1) * TQ, :] = results[c]["y"]
    return out


def kernel(x, Wq, bq, Wk, bk, Wv, bv, **_unused):
    in_maps = make_in_maps(x, Wq, bq, Wk, bk, Wv, bv)
    nc = get_nc()
    res = run_bass_kernel_spmd(nc, in_maps, core_ids=list(range(8)))
    return assemble(res.results)
